# revision 12
# baseline (speedup 1.0000x reference)
import sys, math
sys.path.insert(0, "/opt/trn_rl_repo")
import numpy as np

NB, NCORES = 4, 8
_CACHE = {}


def _consts():
    t = np.arange(512)
    f = np.arange(257)
    ang = 2 * np.pi * np.outer(t, f) / 512.0
    CTre = np.cos(ang)
    CTim = -np.sin(ang)
    w = np.ones(257); w[1:256] = 2.0
    ICre = (w[None, :] * np.cos(ang)) / 512.0
    ICim = (-w[None, :] * np.sin(ang)) / 512.0
    ICT = np.concatenate([ICre.T, ICim.T], axis=0) / 512.0   # [514, 512]
    return CTre.astype(np.float32), CTim.astype(np.float32), ICT.astype(np.float32)


def _build():
    import concourse.bass as bass
    from concourse import bacc
    import concourse.tile as tile
    from concourse import mybir
    from concourse.masks import make_identity

    FP32 = mybir.dt.float32
    U32 = mybir.dt.uint32
    A = mybir.AluOpType
    AF = mybir.ActivationFunctionType
    AX = mybir.AxisListType

    nc = bacc.Bacc("TRN2", target_bir_lowering=False)

    def din(name, shape):
        return nc.dram_tensor(name, shape, FP32, kind="ExternalInput").ap()

    xeT = din("xeT", [NB, 7, 512]); meT = din("meT", [NB, 4, 512])
    xdT = din("xdT", [NB, 7, 512]); mdT = din("mdT", [NB, 4, 512])
    ctre_d = din("ctre", [512, 257]); ctim_d = din("ctim", [512, 257])
    ict_d = din("ict", [514, 512])
    embe_d = din("embe", [21, 512]); tme_d = din("tme", [4, 512])
    embd_d = din("embd", [21, 512]); tmd_d = din("tmd", [4, 512])
    M_d = [din(f"M{i}", [512, 512]) for i in range(4)]
    Wvo_d = [din(f"Wvo{i}", [512, 512]) for i in range(4)]
    ff1_d = [din(f"ff1_{i}", [512, 2048]) for i in range(3)]
    ff2_d = [din(f"ff2_{i}", [2048, 512]) for i in range(3)]
    nw_d = [din(f"nw{i}", [512, 1]) for i in range(2)]
    nb_d = [din(f"nb{i}", [512, 1]) for i in range(2)]
    trw_d = din("trw", [3 * 512, 7])
    prj_d = din("prj", [512, 7]); prjb_d = din("prjb", [7, 1])
    out_d = nc.dram_tensor("out", [NB, 7, 256], FP32, kind="ExternalOutput").ap()

    with tile.TileContext(nc) as tc:
        with tc.tile_pool(name="consts", bufs=1) as cp, \
             tc.tile_pool(name="state", bufs=1) as sp, \
             tc.tile_pool(name="work", bufs=1) as wp, \
             tc.tile_pool(name="ffw", bufs=2) as fwp, \
             tc.tile_pool(name="psA", bufs=4, space="PSUM") as pp, \
             tc.tile_pool(name="psB", bufs=2, space="PSUM") as pq:

            ident = cp.tile([128, 128], FP32, tag="ident")
            make_identity(nc, ident)
            ctre = [cp.tile([128, 257], FP32, tag=f"ctre{i}", name=f"ctre{i}") for i in range(4)]
            ctim = [cp.tile([128, 257], FP32, tag=f"ctim{i}", name=f"ctim{i}") for i in range(4)]
            for i in range(4):
                nc.sync.dma_start(out=ctre[i][:], in_=ctre_d[128 * i:128 * (i + 1), :])
                nc.sync.dma_start(out=ctim[i][:], in_=ctim_d[128 * i:128 * (i + 1), :])
            ict_rows = [(0, 128), (128, 128), (256, 1), (257, 128), (385, 128), (513, 1)]
            ict = []
            for ci, (r0, rn) in enumerate(ict_rows):
                t_ = cp.tile([rn, 512], FP32, tag=f"ictc{ci}", name=f"ictc{ci}")
                nc.sync.dma_start(out=t_[:], in_=ict_d[r0:r0 + rn, :])
                ict.append(t_)
            ones = cp.tile([128, 1], FP32, tag="ones")
            nc.vector.memset(ones[:], 1.0)
            zz536 = cp.tile([128, 536], FP32, tag="zz536")
            nc.vector.memset(zz536[:], 0.0)
            mdiff = []
            for ti in range(4):
                m32 = wp.tile([128, 512], mybir.dt.int32, tag="t512", name=f"m32_{ti}")
                nc.gpsimd.iota(m32[:], pattern=[[-1, 512]], base=ti * 128 + 512,
                               channel_multiplier=1)
                mf = cp.tile([128, 512], FP32, tag=f"mdiff{ti}", name=f"mdiff{ti}")
                nc.vector.tensor_copy(mf[:], m32[:])
                msk = wp.tile([128, 512], FP32, tag="t512", name=f"mdmask{ti}")
                for _ in range(2):
                    nc.vector.tensor_scalar(msk[:], mf[:], 512.0, None, A.is_ge)
                    nc.vector.scalar_tensor_tensor(mf[:], msk[:], -512.0, mf[:], A.mult, A.add)
                mdiff.append(mf)
            embe = cp.tile([21, 512], FP32, tag="embe")
            tme = cp.tile([4, 512], FP32, tag="tme")
            embd = cp.tile([21, 512], FP32, tag="embd")
            tmd = cp.tile([4, 512], FP32, tag="tmd")
            for t_, d_ in [(embe, embe_d), (tme, tme_d), (embd, embd_d), (tmd, tmd_d)]:
                nc.sync.dma_start(out=t_[:], in_=d_[:])
            nw = []; nb_ = []
            for i in range(2):
                for j in range(4):
                    w_ = cp.tile([128, 1], FP32, tag=f"nw{i}{j}", name=f"nw{i}{j}")
                    b_ = cp.tile([128, 1], FP32, tag=f"nb{i}{j}", name=f"nb{i}{j}")
                    nc.sync.dma_start(out=w_[:], in_=nw_d[i][128 * j:128 * (j + 1), :])
                    nc.sync.dma_start(out=b_[:], in_=nb_d[i][128 * j:128 * (j + 1), :])
                    nw.append(w_); nb_.append(b_)
            trw = [cp.tile([128, 7], FP32, tag=f"trw{i}", name=f"trw{i}") for i in range(12)]
            for i in range(12):
                nc.sync.dma_start(out=trw[i][:], in_=trw_d[128 * i:128 * (i + 1), :])
            prj = [cp.tile([128, 7], FP32, tag=f"prj{i}", name=f"prj{i}") for i in range(4)]
            for i in range(4):
                nc.sync.dma_start(out=prj[i][:], in_=prj_d[128 * i:128 * (i + 1), :])
            prjb = cp.tile([7, 1], FP32, tag="prjb")
            nc.sync.dma_start(out=prjb[:], in_=prjb_d[:])

            # ---------- helpers ----------
            def mm4(lhs, rhs, otag, n=512, pool=wp, tags=None):
                outs = []
                for dj in range(4):
                    ps = pp.tile([128, n], FP32, tag="ps", name=f"{otag}ps{dj}")
                    for i in range(4):
                        nc.tensor.matmul(ps[:], lhs[i][:, 128 * dj:128 * (dj + 1)],
                                         rhs[i][:], start=(i == 0), stop=(i == 3))
                    tg = tags[dj] if tags else f"{otag}{dj}"
                    o = pool.tile([128, n], FP32, tag=tg, name=f"{otag}{dj}")
                    nc.scalar.copy(o[:], ps[:])
                    outs.append(o)
                return outs

            def mm4_stream(lhs_dram, rhs, otag, n=512, tags=None, pool=wp):
                outs = []
                for dj in range(4):
                    lw = [fwp.tile([128, 128], FP32, tag=f"lw{i}", name=f"{otag}lw{dj}_{i}")
                          for i in range(4)]
                    for i in range(4):
                        nc.sync.dma_start(out=lw[i][:],
                                          in_=lhs_dram[128 * i:128 * (i + 1),
                                                       128 * dj:128 * (dj + 1)])
                    ps = pp.tile([128, n], FP32, tag="ps", name=f"{otag}ps{dj}")
                    for i in range(4):
                        nc.tensor.matmul(ps[:], lw[i][:], rhs[i][:],
                                         start=(i == 0), stop=(i == 3))
                    tg = tags[dj] if tags else f"{otag}{dj}"
                    o = pool.tile([128, n], FP32, tag=tg, name=f"{otag}{dj}")
                    nc.scalar.copy(o[:], ps[:])
                    outs.append(o)
                return outs

            def transpose4(xt, otag, pool=wp, tags=None):
                outs = []
                for j in range(4):
                    ps = pp.tile([128, 512], FP32, tag="ps", name=f"{otag}ps{j}")
                    for i in range(4):
                        nc.tensor.transpose(ps[:, 128 * i:128 * (i + 1)],
                                            xt[i][:, 128 * j:128 * (j + 1)], ident[:])
                    tg = tags[j] if tags else f"{otag}{j}"
                    o = pool.tile([128, 512], FP32, tag=tg, name=f"{otag}{j}")
                    nc.scalar.copy(o[:], ps[:])
                    outs.append(o)
                return outs

            def decomp1(xt, otag, p=128, want_trend=False, stag=None, ttag=None, spool=wp):
                xp = wp.tile([128, 536], FP32, tag="dcxp", name=f"{otag}xp")
                nc.vector.tensor_copy(xp[0:p, 12:524], xt[0:p, :])
                for c in range(12):
                    nc.scalar.copy(xp[0:p, c:c + 1], xt[0:p, 0:1])
                    nc.scalar.copy(xp[0:p, 524 + c:525 + c], xt[0:p, 511:512])
                cs = wp.tile([128, 536], FP32, tag="dccs", name=f"{otag}cs")
                nc.vector.tensor_tensor_scan(cs[0:p, :], xp[0:p, :], zz536[0:p, :], 0.0,
                                             A.add, A.add)
                tr = wp.tile([128, 512], FP32, tag=(ttag or "dctr0"), name=f"{otag}tr")
                nc.vector.tensor_sub(tr[0:p, 1:512], cs[0:p, 25:536], cs[0:p, 0:511])
                nc.scalar.copy(tr[0:p, 0:1], cs[0:p, 24:25])
                nc.scalar.mul(tr[0:p, :], tr[0:p, :], 1.0 / 25.0)
                s = spool.tile([128, 512], FP32, tag=(stag or f"{otag}s"), name=f"{otag}s")
                nc.vector.tensor_sub(s[0:p, :], xt[0:p, :], tr[0:p, :])
                return (s, tr) if want_trend else s

            def decomp4(xt, otag, stags, spool=wp, want_trend=False):
                res = [decomp1(xt[i], f"{otag}{i}", want_trend=want_trend,
                               stag=stags[i], ttag=(f"dctr{i}" if want_trend else None),
                               spool=spool) for i in range(4)]
                if want_trend:
                    return [r[0] for r in res], [r[1] for r in res]
                return res

            def add4(a, b, otag):
                o = []
                for i in range(4):
                    t_ = wp.tile([128, 512], FP32, tag=f"{otag}{i}", name=f"{otag}{i}")
                    (nc.vector if i % 2 == 0 else nc.gpsimd).tensor_add(t_[:], a[i][:], b[i][:])
                    o.append(t_)
                return o

            def attention(xq, kv_re, kv_im, kv_L2, li, tag):
                qL1 = transpose4(xq, f"{tag}L1", tags=[f"awC{i}" for i in range(4)])
                XFre = mm4(qL1, ctre, f"{tag}xr", n=257, tags=[f"awA{i}" for i in range(4)])
                XFim = mm4(qL1, ctim, f"{tag}xi", n=257, tags=[f"awB{i}" for i in range(4)])
                Yre = mm4_stream(M_d[li], XFre, f"{tag}yr", n=257, tags=[f"awC{i}" for i in range(4)])
                Yim = mm4_stream(M_d[li], XFim, f"{tag}yi", n=257, tags=[f"awD{i}" for i in range(4)])
                Kre = kv_re if kv_re is not None else XFre
                Kim = kv_im if kv_im is not None else XFim
                E1, E2 = [], []
                for i in range(4):
                    t1 = wp.tile([128, 257], FP32, tag="att1", name=f"{tag}t1_{i}")
                    t2 = wp.tile([128, 257], FP32, tag="att2", name=f"{tag}t2_{i}")
                    nc.vector.tensor_mul(t1[:], Yre[i][:, 0:257], Kre[i][:, 0:257])
                    nc.gpsimd.tensor_mul(t2[:], Yim[i][:, 0:257], Kim[i][:, 0:257])
                    t3 = wp.tile([128, 257], FP32, tag="att3", name=f"{tag}t3_{i}")
                    t4 = wp.tile([128, 257], FP32, tag="att4", name=f"{tag}t4_{i}")
                    nc.vector.tensor_mul(t3[:], Yim[i][:, 0:257], Kre[i][:, 0:257])
                    nc.gpsimd.tensor_mul(t4[:], Yre[i][:, 0:257], Kim[i][:, 0:257])
                    # write E1 into Yre slot, E2 into Yim slot (they are dead now)
                    nc.vector.tensor_add(Yre[i][:, 0:257], t1[:], t2[:])
                    nc.gpsimd.tensor_sub(Yim[i][:, 0:257], t3[:], t4[:])
                    E1.append(Yre[i]); E2.append(Yim[i])
                chunks = [(E1, 0, 128, ict[0]), (E1, 128, 128, ict[1]),
                          (E1, 256, 1, ict[2]), (E2, 0, 128, ict[3]),
                          (E2, 128, 128, ict[4]), (E2, 256, 1, ict[5])]
                pcs = []
                for ci, (E, f0, fn, _) in enumerate(chunks):
                    ps = pq.tile([128, 1], FP32, tag="ps1", name=f"{tag}P{ci}")
                    for i in range(4):
                        nc.tensor.matmul(ps[0:fn, :], E[i][:, f0:f0 + fn], ones[:],
                                         start=(i == 0), stop=(i == 3))
                    o = wp.tile([128, 1], FP32, tag=f"atPo{ci}", name=f"{tag}Po{ci}")
                    nc.scalar.copy(o[0:fn, :], ps[0:fn, :])
                    pcs.append(o)
                psmc = pq.tile([1, 512], FP32, tag="ps1", name=f"{tag}mcps")
                for ci, (E, f0, fn, it) in enumerate(chunks):
                    nc.tensor.matmul(psmc[:], pcs[ci][0:fn, :], it[0:fn, :],
                                     start=(ci == 0), stop=(ci == 5))
                rows = wp.tile([8, 512], FP32, tag="rows8", name=f"{tag}rows")
                nc.scalar.copy(rows[0:1, :], psmc[:])
                mcv = wp.tile([1, 8], FP32, tag="atmcv", name=f"{tag}mcv")
                mci = wp.tile([1, 8], U32, tag="atmci", name=f"{tag}mci")
                nc.vector.max_with_indices(mcv[:], mci[:], rows[0:1, :])
                sm = wp.tile([1, 6], FP32, tag="atsm", name=f"{tag}sm")
                nc.vector.tensor_scalar(sm[:], mcv[:, 0:6], mcv[:, 0:1], None, A.subtract)
                nc.scalar.activation(sm[:], sm[:], AF.Exp)
                ssum = wp.tile([1, 1], FP32, tag="atss", name=f"{tag}ss")
                nc.vector.tensor_reduce(ssum[:], sm[:], axis=AX.X, op=A.add)
                nc.vector.reciprocal(ssum[:], ssum[:])
                nc.vector.tensor_scalar(sm[:], sm[:], ssum[:, 0:1], None, A.mult)
                idxf = wp.tile([1, 8], FP32, tag="atif", name=f"{tag}if")
                nc.vector.tensor_copy(idxf[:], mci[:])
                dbc = wp.tile([128, 8], FP32, tag="atdb", name=f"{tag}db")
                tbc = wp.tile([128, 6], FP32, tag="attb", name=f"{tag}tb")
                nc.gpsimd.partition_broadcast(dbc[:], idxf[:])
                nc.gpsimd.partition_broadcast(tbc[:], sm[:])
                G2 = []
                for ti in range(4):
                    g = wp.tile([128, 512], FP32, tag=f"awA{ti}", name=f"{tag}g{ti}")
                    nc.vector.memset(g[:], 0.0)
                    msk = wp.tile([128, 512], FP32, tag="t512", name=f"{tag}gm{ti}")
                    for k in range(6):
                        (nc.vector if k % 2 else nc.gpsimd).tensor_scalar(
                            msk[:], mdiff[ti][:], dbc[:, k:k + 1], None, A.is_equal)
                        nc.vector.scalar_tensor_tensor(g[:], msk[:], tbc[:, k:k + 1],
                                                       g[:], A.mult, A.add)
                    G2.append(g)
                # voL1[t-chunk dj] = sum_i kvL2[i][:,t-chunk].T @ Wvo[i]
                voL1 = [wp.tile([128, 512], FP32, tag=f"awB{dj}", name=f"{tag}vo{dj}")
                        for dj in range(4)]
                vps = [pp.tile([128, 512], FP32, tag="ps", name=f"{tag}vps{dj}")
                       for dj in range(4)]
                for i in range(4):
                    wv = fwp.tile([128, 512], FP32, tag="f2w", name=f"{tag}wv{i}")
                    nc.sync.dma_start(out=wv[:], in_=Wvo_d[li][128 * i:128 * (i + 1), :])
                    for dj in range(4):
                        nc.tensor.matmul(vps[dj][:], kv_L2[i][:, 128 * dj:128 * (dj + 1)],
                                         wv[:], start=(i == 0), stop=(i == 3))
                for dj in range(4):
                    nc.scalar.copy(voL1[dj][:], vps[dj][:])
                return mm4(voL1, G2, f"{tag}ao", tags=[f"awC{i}" for i in range(4)])

            def ffn(s, li, tag):
                acc = [pp.tile([128, 512], FP32, tag="ps", name=f"{tag}acc{dj}")
                       for dj in range(4)]
                for k in range(16):
                    f1w = [fwp.tile([128, 128], FP32, tag=f"f1w{i}", name=f"{tag}f1w{k}_{i}")
                           for i in range(4)]
                    for i in range(4):
                        nc.sync.dma_start(out=f1w[i][:],
                                          in_=ff1_d[li][128 * i:128 * (i + 1),
                                                        128 * k:128 * (k + 1)])
                    ps = pq.tile([128, 512], FP32, tag="psf", name=f"{tag}psF{k}")
                    for i in range(4):
                        nc.tensor.matmul(ps[:], f1w[i][:], s[i][:],
                                         start=(i == 0), stop=(i == 3))
                    h = fwp.tile([128, 512], FP32, tag="h", name=f"{tag}h{k}", bufs=3)
                    nc.scalar.activation(h[:], ps[:], AF.Gelu)
                    f2w = fwp.tile([128, 512], FP32, tag="f2w", name=f"{tag}f2w{k}")
                    nc.sync.dma_start(out=f2w[:], in_=ff2_d[li][128 * k:128 * (k + 1), :])
                    for dj in range(4):
                        nc.tensor.matmul(acc[dj][:], f2w[:, 128 * dj:128 * (dj + 1)],
                                         h[:], start=(k == 0), stop=(k == 15))
                y = []
                for dj in range(4):
                    o = wp.tile([128, 512], FP32, tag=f"resid{dj}", name=f"{tag}y{dj}")
                    nc.vector.tensor_add(o[:], acc[dj][:], s[dj][:])
                    y.append(o)
                return y

            def my_ln(x, ni, otag, stags, polish=True):
                r_mu = wp.tile([1, 512], FP32, tag="lnr1", name=f"{otag}rmu")
                r_va = wp.tile([1, 512], FP32, tag="lnr2", name=f"{otag}rva")
                r_t = wp.tile([1, 512], FP32, tag="lnr3", name=f"{otag}rt")
                r_sd = wp.tile([1, 512], FP32, tag="lnr4", name=f"{otag}rsd")
                r_rs = wp.tile([1, 512], FP32, tag="lnr5", name=f"{otag}rrs")
                mu, var, t, sd, rs = r_mu[:], r_va[:], r_t[:], r_sd[:], r_rs[:]
                musp = pq.tile([1, 512], FP32, tag="ps1", name=f"{otag}psmu")
                for i in range(4):
                    nc.tensor.matmul(musp[:], ones[:], x[i][:], start=(i == 0), stop=(i == 3))
                nc.scalar.mul(mu, musp[:], 1.0 / 512.0)
                vsp = pq.tile([1, 512], FP32, tag="ps1", name=f"{otag}psv")
                for i in range(4):
                    sq = wp.tile([128, 512], FP32, tag="t512", name=f"{otag}sq{i}")
                    nc.vector.tensor_mul(sq[:], x[i][:], x[i][:])
                    nc.tensor.matmul(vsp[:], ones[:], sq[:], start=(i == 0), stop=(i == 3))
                nc.scalar.mul(var, vsp[:], 1.0 / 512.0)
                nc.vector.tensor_mul(t, mu, mu)
                nc.vector.tensor_sub(var, var, t)
                nc.vector.tensor_scalar_add(var, var, 1e-5)
                nc.scalar.activation(sd, var, AF.Sqrt)
                nc.vector.reciprocal(rs, sd)
                if polish:
                    nc.vector.tensor_mul(t, rs, rs)
                    nc.vector.tensor_mul(t, t, var)
                    nc.vector.tensor_scalar(t, t, -0.5, 1.5, A.mult, A.add)
                    nc.vector.tensor_mul(rs, rs, t)
                mub = wp.tile([128, 512], FP32, tag="awD0", name=f"{otag}mub")
                rsb = wp.tile([128, 512], FP32, tag="awD1", name=f"{otag}rsb")
                nc.gpsimd.partition_broadcast(mub[:], mu)
                nc.gpsimd.partition_broadcast(rsb[:], rs)
                out = []
                for i in range(4):
                    xn = sp.tile([128, 512], FP32, tag=stags[i], name=f"{otag}{i}")
                    nc.vector.tensor_sub(xn[:], x[i][:], mub[:])
                    nc.gpsimd.tensor_mul(xn[:], xn[:], rsb[:])
                    nc.vector.tensor_scalar(xn[:], xn[:], nw[4 * ni + i][:, 0:1],
                                            nb_[4 * ni + i][:, 0:1], A.mult, A.add)
                    m2 = wp.tile([128, 1], FP32, tag="lnm2", name=f"{otag}m2_{i}")
                    nc.vector.tensor_reduce(m2[:], xn[:], axis=AX.X, op=A.add)
                    nc.scalar.mul(m2[:], m2[:], 1.0 / 512.0)
                    nc.vector.tensor_scalar(xn[:], xn[:], m2[:, 0:1], None, A.subtract)
                    out.append(xn)
                return out

            def embed(src7, marks, wconv, wtime, stags, otag):
                x3 = wp.tile([21, 512], FP32, tag="embx3", name=f"{otag}x3")
                nc.vector.tensor_copy(x3[0:7, 1:512], src7[0:7, 0:511])
                nc.vector.tensor_copy(x3[0:7, 0:1], src7[0:7, 511:512])
                nc.sync.dma_start(out=x3[7:14, 0:512], in_=src7[0:7, :])
                nc.sync.dma_start(out=x3[14:21, 0:511], in_=src7[0:7, 1:512])
                nc.sync.dma_start(out=x3[14:21, 511:512], in_=src7[0:7, 0:1])
                outs = []
                for dj in range(4):
                    ps = pp.tile([128, 512], FP32, tag="ps", name=f"{otag}ps{dj}")
                    nc.tensor.matmul(ps[:], wconv[:, 128 * dj:128 * (dj + 1)], x3[:],
                                     start=True, stop=False)
                    nc.tensor.matmul(ps[:], wtime[:, 128 * dj:128 * (dj + 1)], marks[:],
                                     start=False, stop=True)
                    o = sp.tile([128, 512], FP32, tag=stags[dj], name=f"{otag}{dj}")
                    nc.scalar.copy(o[:], ps[:])
                    outs.append(o)
                return outs

            ST_E = [f"stE{i}" for i in range(4)]
            ST_D = [f"stD{i}" for i in range(4)]
            ST_N = [f"stN{i}" for i in range(4)]
            ST_A = [f"stA{i}" for i in range(4)]   # dec s1 / s3
            ST_B = [f"stB{i}" for i in range(4)]   # dec s2

            # ================= per-batch =================
            for b in range(NB):
                xe_t = wp.tile([7, 512], FP32, tag="inxe", name=f"b{b}xe")
                me_t = wp.tile([4, 512], FP32, tag="inme", name=f"b{b}me")
                xd_t = wp.tile([7, 512], FP32, tag="inxe", name=f"b{b}xd")
                md_t = wp.tile([4, 512], FP32, tag="inme", name=f"b{b}md")
                xe, me, xd, md = xe_t[:], me_t[:], xd_t[:], md_t[:]
                nc.sync.dma_start(out=xe, in_=xeT[b])
                nc.sync.dma_start(out=me, in_=meT[b])
                nc.sync.dma_start(out=xd, in_=xdT[b])
                nc.sync.dma_start(out=md, in_=mdT[b])

                sea_e, tr_e = decomp1(xe, f"b{b}d7", p=7, want_trend=True, stag="d7s", ttag="dctr0")
                seaini_t = wp.tile([7, 512], FP32, tag="seaini", name=f"b{b}sea")
                seaini = seaini_t[:]
                mean_e = wp.tile([7, 1], FP32, tag="mean7", name=f"b{b}mn")
                nc.vector.tensor_reduce(mean_e[:], xe, axis=AX.X, op=A.add)
                nc.scalar.mul(mean_e[:], mean_e[:], 1.0 / 512.0)
                trini = sp.tile([7, 512], FP32, tag="trini", name=f"b{b}trini")
                nc.vector.tensor_copy(trini[0:7, 0:256], tr_e[0:7, 256:512])
                nc.vector.memset(trini[0:7, 256:512], 0.0)
                nc.vector.tensor_scalar(trini[0:7, 256:512], trini[0:7, 256:512],
                                        mean_e[:, 0:1], None, A.add)
                nc.vector.tensor_copy(seaini[:, 0:256], sea_e[0:7, 256:512])
                nc.vector.memset(seaini[:, 256:512], 0.0)

                enc = embed(xe, me, embe, tme, ST_E, f"b{b}ee")
                dec = embed(seaini, md, embd, tmd, ST_D, f"b{b}ed")

                for l in range(2):
                    ao = attention(enc, None, None, enc, l, f"b{b}e{l}")
                    x1 = add4(enc, ao, "resid")
                    s1 = decomp4(x1, f"b{b}es{l}", stags=[f"es1_{i}" for i in range(4)])
                    y = ffn(s1, l, f"b{b}e{l}")
                    enc = decomp4(y, f"b{b}en{l}", stags=ST_E, spool=sp)
                encN = my_ln(enc, 0, f"b{b}eN", ST_N)
                encL1 = transpose4(encN, f"b{b}ecl", tags=[f"awC{i}" for i in range(4)])
                EFre = mm4(encL1, ctre, f"b{b}efr", n=257, pool=sp,
                           tags=[f"stFr{i}" for i in range(4)])
                EFim = mm4(encL1, ctim, f"b{b}efi", n=257, pool=sp,
                           tags=[f"stFi{i}" for i in range(4)])

                ao = attention(dec, None, None, dec, 2, f"b{b}ds")
                x1 = add4(dec, ao, "resid")
                s1, t1 = decomp4(x1, f"b{b}d1", stags=ST_A, spool=sp, want_trend=True)
                tsum = []
                for i in range(4):
                    ts_ = sp.tile([128, 514], FP32, tag=f"tsum{i}", name=f"b{b}ts{i}")
                    nc.vector.tensor_copy(ts_[:, 1:513], t1[i][:])
                    tsum.append(ts_)
                ao2 = attention(s1, EFre, EFim, encN, 3, f"b{b}cr")
                x2 = add4(s1, ao2, "resid")
                s2, t2 = decomp4(x2, f"b{b}d2", stags=ST_B, spool=sp, want_trend=True)
                for i in range(4):
                    nc.vector.tensor_add(tsum[i][:, 1:513], tsum[i][:, 1:513], t2[i][:])
                y = ffn(s2, 2, f"b{b}df")
                s3, t3 = decomp4(y, f"b{b}d3", stags=ST_A, spool=sp, want_trend=True)
                for i in range(4):
                    nc.vector.tensor_add(tsum[i][:, 1:513], tsum[i][:, 1:513], t3[i][:])
                    nc.scalar.copy(tsum[i][:, 0:1], tsum[i][:, 512:513])
                    nc.scalar.copy(tsum[i][:, 513:514], tsum[i][:, 1:2])
                decN = my_ln(s3, 1, f"b{b}dN", ST_B, polish=False)

                pstr = pq.tile([7, 512], FP32, tag="ps1", name=f"b{b}pstr")
                first = True
                for kap in range(3):
                    for i in range(4):
                        nc.tensor.matmul(pstr[:], trw[4 * kap + i][:],
                                         tsum[i][:, kap:kap + 512],
                                         start=first, stop=(kap == 2 and i == 3))
                        first = False
                trend_t = wp.tile([7, 512], FP32, tag="trendrow", name=f"b{b}trd")
                trend = trend_t[:]
                nc.vector.tensor_add(trend, pstr[0:7, :], trini[0:7, :])

                pspj = pq.tile([7, 512], FP32, tag="ps1", name=f"b{b}pspj")
                for i in range(4):
                    nc.tensor.matmul(pspj[:], prj[i][:], decN[i][:],
                                     start=(i == 0), stop=(i == 3))
                outT_t = wp.tile([7, 512], FP32, tag="seaini", name=f"b{b}ot")
                outT = outT_t[:]
                nc.vector.tensor_add(outT, pspj[0:7, :], trend)
                nc.vector.tensor_scalar(outT, outT, prjb[:, 0:1], None, A.add)
                nc.sync.dma_start(out=out_d[b], in_=outT[0:7, 256:512])

    nc.finalize()
    return nc


def _prep(inputs):
    p = inputs["params"]
    f32 = np.float32

    def n64(a):
        return np.asarray(a, dtype=np.float64)

    xeT = np.ascontiguousarray(np.asarray(inputs["x_enc"], f32).transpose(0, 2, 1))
    meT = np.ascontiguousarray(np.asarray(inputs["x_mark_enc"], f32).transpose(0, 2, 1))
    xdT = np.ascontiguousarray(np.asarray(inputs["x_dec"], f32).transpose(0, 2, 1))
    mdT = np.ascontiguousarray(np.asarray(inputs["x_mark_dec"], f32).transpose(0, 2, 1))
    CTre, CTim, ICT = _consts()

    def attn_mats(ap):
        return ((n64(ap["q_w"]) @ n64(ap["k_w"]).T).astype(f32),
                (n64(ap["v_w"]) @ n64(ap["o_w"])).astype(f32))

    enc_l = p["enc_layers"]; dec_l = p["dec_layers"][0]
    attns = [attn_mats(enc_l[0]["attn"]), attn_mats(enc_l[1]["attn"]),
             attn_mats(dec_l["self"]), attn_mats(dec_l["cross"])]
    wmap = {
        "ctre": CTre, "ctim": CTim, "ict": ICT,
        "embe": np.asarray(p["enc_emb"]["token"], f32).reshape(21, 512),
        "tme": np.asarray(p["enc_emb"]["time"], f32),
        "embd": np.asarray(p["dec_emb"]["token"], f32).reshape(21, 512),
        "tmd": np.asarray(p["dec_emb"]["time"], f32),
        "trw": np.asarray(dec_l["trend_w"], f32).reshape(3 * 512, 7),
        "prj": np.asarray(p["proj_w"], f32),
        "prjb": np.asarray(p["proj_b"], f32).reshape(7, 1),
        "nw0": np.asarray(p["enc_norm"]["w"], f32).reshape(512, 1),
        "nb0": np.asarray(p["enc_norm"]["b"], f32).reshape(512, 1),
        "nw1": np.asarray(p["dec_norm"]["w"], f32).reshape(512, 1),
        "nb1": np.asarray(p["dec_norm"]["b"], f32).reshape(512, 1),
    }
    for i, (M, Wvo) in enumerate(attns):
        wmap[f"M{i}"] = M; wmap[f"Wvo{i}"] = Wvo
    for i, lp in enumerate([enc_l[0], enc_l[1], dec_l]):
        wmap[f"ff1_{i}"] = np.asarray(lp["ff1"], f32)
        wmap[f"ff2_{i}"] = np.asarray(lp["ff2"], f32)
    in_maps = []
    for c in range(NCORES):
        s = slice(NB * c, NB * (c + 1))
        m = dict(wmap)
        m["xeT"], m["meT"], m["xdT"], m["mdT"] = xeT[s], meT[s], xdT[s], mdT[s]
        in_maps.append(m)
    return in_maps


def kernel(x_enc, x_mark_enc, x_dec, x_mark_dec, params):
    from concourse.bass_utils import run_bass_kernel_spmd
    in_maps = _prep({"x_enc": x_enc, "x_mark_enc": x_mark_enc, "x_dec": x_dec,
                     "x_mark_dec": x_mark_dec, "params": params})
    if "nc" not in _CACHE:
        _CACHE["nc"] = _build()
    res = run_bass_kernel_spmd(_CACHE["nc"], in_maps, list(range(NCORES)))
    outs = [res.results[c]["out"].transpose(0, 2, 1) for c in range(NCORES)]
    return np.concatenate(outs, axis=0).astype(np.float32)
